# revision 1
# baseline (speedup 1.0000x reference)
"""VQ codebook encode+decode kernel for Trainium2 (8 NeuronCores, SPMD).

Problem: images (65536, 256) f32, mu (256, 512) f32.
  kmax[b] = argmin_k ||images[b] - mu[:,k]||^2  (ties -> first k)
  recon   = mu.T[kmax]                          -> (65536, 256) f32

Strategy (data-parallel over batch, 8192 rows/core):
  argmin_k dist2 == argmax_k nscore,  nscore[b,k] = 2*x@mu - m2[k]
  (the x2[b] term is row-constant; dropping it provably does not change the
  argmin: measured min top-2 gap over all rows is 1.1e-5, far above noise).

  Precision: PE fp16 matmuls with hi/lo split (x = xh + xl, m = mh + ml,
  xh@mh + xh@ml + xl@mh accumulated in fp32 PSUM). fp16xfp16 products are
  exact in fp32; total error ~1e-7, so the argmax matches fp32/fp64 exactly.
  m2 enters as an extra 2-row accumulation (ones @ [-m2_hi; -m2_lo]).

  Per 128-row tile: 7 accumulating matmuls -> PSUM [128,512]; ScalarE copies
  PSUM->SBUF; VectorE max8 + max_index -> argmax index; gpsimd indirect DMA
  gathers mu.T rows from DRAM; HWDGE stores the [128,256] recon tile.

Host side packs per-core inputs (transpose + fp16 split) with numpy.
"""

import numpy as np

B_FULL = 65536
G = 256
K = 512
NCORES = 8
BS = B_FULL // NCORES  # 8192 rows per core
NT = BS // 128  # 64 row-tiles per core

_CACHE = {}


def _split_excess_waits(nc, max_waits=1):
    """Walrus in this container rejects instructions with more than ~2 sync
    waits (e.g. Tile's kernel-tail Drain carries 19). Hoist excess waits onto
    freshly inserted same-engine NoOps directly before the offender — engine
    program order makes sequential waiting equivalent to the AND of all
    conditions."""
    import concourse.mybir as mybir

    for fn in nc.m.functions:
        for blk in fn.blocks:
            newlist = []
            for inst in blk.instructions:
                si = inst.sync_info
                waits = list(si.on_wait) if si is not None else []
                if len(waits) > max_waits:
                    head, tail = waits[:-max_waits], waits[-max_waits:]
                    for i in range(0, len(head), max_waits):
                        chunk = head[i:i + max_waits]
                        nop = mybir.InstNoOp(
                            name=f"{inst.name}_waitsplit{i}",
                            engine=inst.engine,
                            sync_info=mybir.SyncInfo(
                                on_wait=chunk, on_update=[]
                            ),
                        )
                        newlist.append(nop)
                    si.on_wait = tail
                newlist.append(inst)
            blk.instructions = newlist
    return nc


def _build_bass(ntiles=NT):
    import concourse.bass as bass
    import concourse.mybir as mybir
    import concourse.tile as tile

    nc = bass.Bass()
    dt = mybir.dt

    # [c_chunk, g_within_chunk, tile_j, hi/lo, b_within_tile]
    imt = nc.dram_tensor("imt", [2, 128, ntiles, 2, 128], dt.float16,
                         kind="ExternalInput")
    muw = nc.dram_tensor("muw", [2, 2, 128, K], dt.float16, kind="ExternalInput")
    biasw = nc.dram_tensor("biasw", [2, K], dt.float16, kind="ExternalInput")
    onesw = nc.dram_tensor("onesw", [2, 128], dt.float16, kind="ExternalInput")
    gtab = nc.dram_tensor("gtab", [K, G], dt.float32, kind="ExternalInput")
    out = nc.dram_tensor("out", [ntiles * 128, G], dt.float32,
                         kind="ExternalOutput")

    with tile.TileContext(nc) as tc:
        with (
            tc.tile_pool(name="w", bufs=1) as wpool,
            tc.tile_pool(name="x", bufs=6) as xpool,
            tc.tile_pool(name="ps", bufs=4, space="PSUM") as pspool,
            tc.tile_pool(name="s", bufs=6) as spool,
            tc.tile_pool(name="r", bufs=6) as rpool,
        ):
            mw = [[wpool.tile([128, K], dt.float16, tag=f"mw{c}{h}",
                              name=f"mw{c}{h}")
                   for h in range(2)] for c in range(2)]
            for c in range(2):
                for h in range(2):
                    nc.sync.dma_start(mw[c][h][:], muw[c, h, :, :])
            bias_sb = wpool.tile([2, K], dt.float16, tag="bias")
            nc.sync.dma_start(bias_sb[:], biasw[:])
            ones_sb = wpool.tile([2, 128], dt.float16, tag="ones")
            nc.sync.dma_start(ones_sb[:], onesw[:])

            for j in range(ntiles):
                xt0 = xpool.tile([128, 256], dt.float16, tag="xt0")
                xt1 = xpool.tile([128, 256], dt.float16, tag="xt1")
                nc.sync.dma_start(xt0[:], imt[0, :, j, :, :])
                nc.sync.dma_start(xt1[:], imt[1, :, j, :, :])

                ps = pspool.tile([128, K], dt.float32, tag="ps")
                # hi@hi, hi@lo, lo@hi for each of the two 128-row g chunks,
                # then the 2-row bias matmul adds -m2 (hi+lo).
                nc.tensor.matmul(ps[:], xt0[:, 0:128], mw[0][0][:],
                                 start=True, stop=False)
                nc.tensor.matmul(ps[:], xt0[:, 0:128], mw[0][1][:],
                                 start=False, stop=False)
                nc.tensor.matmul(ps[:], xt0[:, 128:256], mw[0][0][:],
                                 start=False, stop=False)
                nc.tensor.matmul(ps[:], xt1[:, 0:128], mw[1][0][:],
                                 start=False, stop=False)
                nc.tensor.matmul(ps[:], xt1[:, 0:128], mw[1][1][:],
                                 start=False, stop=False)
                nc.tensor.matmul(ps[:], xt1[:, 128:256], mw[1][0][:],
                                 start=False, stop=False)
                nc.tensor.matmul(ps[:], ones_sb[:], bias_sb[:],
                                 start=False, stop=True)

                score = spool.tile([128, K], dt.float32, tag="score")
                nc.scalar.copy(score[:], ps[:])

                mx8 = spool.tile([128, 8], dt.float32, tag="mx8")
                nc.vector.max(out=mx8[:], in_=score[:])
                idx = spool.tile([128, 8], dt.uint32, tag="idx")
                nc.vector.max_index(idx[:], mx8[:], score[:])

                rec = rpool.tile([128, G], dt.float32, tag="rec")
                nc.gpsimd.indirect_dma_start(
                    out=rec[:],
                    out_offset=None,
                    in_=gtab[:],
                    in_offset=bass.IndirectOffsetOnAxis(ap=idx[:, 0:1], axis=0),
                )
                nc.sync.dma_start(out[bass.ts(j, 128), :], rec[:])

    return _split_excess_waits(nc)


def _prep_shared(mu):
    mu64 = np.asarray(mu, np.float64)
    mu2 = 2.0 * mu64  # (G, K)
    mh = mu2.astype(np.float16)
    ml = (mu2 - mh.astype(np.float64)).astype(np.float16)
    muw = np.zeros([2, 2, 128, K], np.float16)
    for c in range(2):
        muw[c, 0] = mh[c * 128:(c + 1) * 128]
        muw[c, 1] = ml[c * 128:(c + 1) * 128]

    m2 = (mu64 * mu64).sum(0)  # (K,)
    bh = (-m2).astype(np.float16)
    bl = (-m2 - bh.astype(np.float64)).astype(np.float16)
    biasw = np.stack([bh, bl], 0)  # (2, K)

    onesw = np.ones([2, 128], np.float16)
    gtab = np.ascontiguousarray(np.asarray(mu, np.float32).T)  # (K, G)
    return muw, biasw, onesw, gtab


def _prep_core_images(shard):
    # shard: (BS, G) f32 -> imt [2, 128, nt, 2, 128] fp16
    nt = shard.shape[0] // 128
    x64 = shard.astype(np.float64)
    xh = x64.astype(np.float16)
    xl = (x64 - xh.astype(np.float64)).astype(np.float16)
    xhT = np.ascontiguousarray(xh.T).reshape(2, 128, nt, 128)
    xlT = np.ascontiguousarray(xl.T).reshape(2, 128, nt, 128)
    return np.ascontiguousarray(np.stack([xhT, xlT], axis=3))


def kernel(images, mu, trace=False):
    from concourse import bass_utils

    images = np.asarray(images, np.float32)
    mu = np.asarray(mu, np.float32)

    if "nc" not in _CACHE:
        _CACHE["nc"] = _build_bass()
    nc = _CACHE["nc"]

    muw, biasw, onesw, gtab = _prep_shared(mu)
    in_maps = []
    for i in range(NCORES):
        shard = images[i * BS:(i + 1) * BS]
        in_maps.append({
            "imt": _prep_core_images(shard),
            "muw": muw,
            "biasw": biasw,
            "onesw": onesw,
            "gtab": gtab,
        })

    res = bass_utils.run_bass_kernel_spmd(
        nc, in_maps, core_ids=list(range(NCORES)), trace=trace
    )
    _CACHE["last_results"] = res
    outs = [r["out"] for r in res.results]
    return np.concatenate(outs, axis=0)



# revision 3
# speedup vs baseline: 1.1663x; 1.1663x over previous
"""VQ codebook encode+decode kernel for Trainium2 (8 NeuronCores, SPMD).

Problem: images (65536, 256) f32, mu (256, 512) f32.
  kmax[b] = argmin_k ||images[b] - mu[:,k]||^2  (ties -> first k)
  recon   = mu.T[kmax]                          -> (65536, 256) f32

Strategy (data-parallel over batch, 8192 rows/core, 64 row-tiles):
  argmin_k dist2 == argmax_k nscore,  nscore[b,k] = 2*x@mu - m2[k]
  (x2[b] is row-constant; dropping it does not change the argmin).

  Encode: 3 accumulating fp32r matmuls per 128-row tile (two 128-g chunks of
  x^T @ 2mu, plus a contract-1 bias pass ones @ -m2) -> PSUM [128,512] f32.
  fp32r runs at full PE rate for wide outputs but carries ~2e-4 accumulation
  noise, so scores are approximate; see the host patch below.

  ACT copies PSUM->SBUF; DVE max8 + max_index give the per-row argmax and the
  top-8 values. gpsimd indirect DMA gathers fp16 codebook rows (4 tiles per
  gather), and the fp16 recon tiles are stored; host upcasts to f32.

  Correctness: device top-2 score gap is exported (mx8). Rows whose gap is
  below TAU (~6.5 sigma of the measured fp32r noise) are exactly rescored on
  the host in fp64 and patched (a few hundred of 65536 rows). All other rows
  provably keep the exact argmax; the remaining error is the fp16 rounding of
  the gathered codebook values (~1e-4 relative, tolerance is 2e-2).

Host side packs per-core inputs (transpose to g-major) with numpy.
"""

import numpy as np

B_FULL = 65536
G = 256
K = 512
NCORES = 8
BS = B_FULL // NCORES  # 8192 rows per core
NT = BS // 128  # 64 row-tiles per core
TAU = 1.5e-3  # host-rescore threshold on device top-2 gap

_CACHE = {}


def _split_excess_waits(nc, max_waits=1):
    """Walrus in this container rejects instructions with more than ~2 sync
    waits (e.g. Tile's kernel-tail Drain carries 19). Hoist excess waits onto
    freshly inserted same-engine NoOps directly before the offender — engine
    program order makes sequential waiting equivalent to the AND of all
    conditions."""
    import concourse.mybir as mybir

    for fn in nc.m.functions:
        for blk in fn.blocks:
            newlist = []
            for inst in blk.instructions:
                si = inst.sync_info
                waits = list(si.on_wait) if si is not None else []
                if len(waits) > max_waits:
                    head, tail = waits[:-max_waits], waits[-max_waits:]
                    for i in range(0, len(head), max_waits):
                        chunk = head[i:i + max_waits]
                        nop = mybir.InstNoOp(
                            name=f"{inst.name}_waitsplit{i}",
                            engine=inst.engine,
                            sync_info=mybir.SyncInfo(
                                on_wait=chunk, on_update=[]
                            ),
                        )
                        newlist.append(nop)
                    si.on_wait = tail
                newlist.append(inst)
            blk.instructions = newlist
    return nc


def _build_bass(ntiles=NT):
    import concourse.bass as bass
    import concourse.mybir as mybir
    import concourse.tile as tile

    nc = bass.Bass()
    dt = mybir.dt

    xt = nc.dram_tensor("xt", [2, 128, ntiles * 128], dt.float32r,
                        kind="ExternalInput")
    mu2 = nc.dram_tensor("mu2", [2, 128, K], dt.float32r, kind="ExternalInput")
    onesb = nc.dram_tensor("onesb", [1, 128], dt.float32r, kind="ExternalInput")
    negm2 = nc.dram_tensor("negm2", [1, K], dt.float32r, kind="ExternalInput")
    gtab16 = nc.dram_tensor("gtab16", [K, G], dt.float16, kind="ExternalInput")
    out16 = nc.dram_tensor("out16", [ntiles * 128, G], dt.float16,
                           kind="ExternalOutput")
    mxv = nc.dram_tensor("mxv", [ntiles // 8, 128, 8, 8], dt.float32,
                         kind="ExternalOutput")
    idxv = nc.dram_tensor("idxv", [ntiles // 8, 128, 8, 8], dt.uint32,
                          kind="ExternalOutput")

    with tile.TileContext(nc) as tc:
        with (
            tc.tile_pool(name="w", bufs=1) as wpool,
            tc.tile_pool(name="x", bufs=2) as xpool,
            tc.tile_pool(name="ps", bufs=4, space="PSUM") as pspool,
            tc.tile_pool(name="s", bufs=4) as spool,
            tc.tile_pool(name="m", bufs=2) as mpool,
            tc.tile_pool(name="g", bufs=3) as gpool,
        ):
            mu_sb = [wpool.tile([128, K], dt.float32r, tag=f"mu{c}", name=f"mu{c}")
                     for c in range(2)]
            for c in range(2):
                nc.sync.dma_start(mu_sb[c][:], mu2[c, :, :])
            ones_sb = wpool.tile([1, 128], dt.float32r, tag="ones")
            nc.sync.dma_start(ones_sb[:], onesb[:])
            negm2_sb = wpool.tile([1, K], dt.float32r, tag="negm2")
            nc.sync.dma_start(negm2_sb[:], negm2[:])

            for j8 in range(ntiles // 8):
                # 8-tile batched input loads (one DMA per g-chunk)
                x8 = [xpool.tile([128, 8 * 128], dt.float32r, tag=f"x8c{c}",
                                 name=f"x8c{c}")
                      for c in range(2)]
                for c in range(2):
                    nc.sync.dma_start(
                        x8[c][:], xt[c, :, j8 * 1024:(j8 + 1) * 1024])

                mx8t = mpool.tile([128, 8, 8], dt.float32, tag="mx8t")
                idx8t = mpool.tile([128, 8, 8], dt.uint32, tag="idx8t")

                for h in range(2):  # two 4-tile half-groups
                    rec = gpool.tile([128, 4, G], dt.float16, tag="rec")
                    for i4 in range(4):
                        i = h * 4 + i4
                        ps = pspool.tile([128, K], dt.float32, tag="ps")
                        nc.tensor.matmul(ps[:], x8[0][:, i * 128:(i + 1) * 128],
                                         mu_sb[0][:], start=True, stop=False)
                        nc.tensor.matmul(ps[:], x8[1][:, i * 128:(i + 1) * 128],
                                         mu_sb[1][:], start=False, stop=False)
                        nc.tensor.matmul(ps[:], ones_sb[:], negm2_sb[:],
                                         start=False, stop=True)

                        score = spool.tile([128, K], dt.float32, tag="score")
                        nc.scalar.copy(out=score[:], in_=ps[:])

                        nc.vector.max(out=mx8t[:, i, :], in_=score[:])
                        nc.vector.max_index(idx8t[:, i, :], mx8t[:, i, :],
                                            score[:])

                        # per-tile gather (multi-index gathers misbehave on HW)
                        nc.gpsimd.indirect_dma_start(
                            out=rec[:, i4, :], out_offset=None, in_=gtab16[:],
                            in_offset=bass.IndirectOffsetOnAxis(
                                ap=idx8t[:, i, 0:1], axis=0),
                        )

                    j4 = j8 * 2 + h
                    ov = out16[bass.ts(j4, 512), :].rearrange(
                        "(i p) g -> p i g", i=4)
                    nc.sync.dma_start(ov, rec[:])

                # 8-tile batched exports for the host patch (Pool queue)
                nc.gpsimd.dma_start(mxv[j8, :, :, :], mx8t[:])
                nc.gpsimd.dma_start(idxv[j8, :, :, :], idx8t[:])

    return _split_excess_waits(nc)


def _prep_shared(mu):
    mu64 = np.asarray(mu, np.float64)
    mu2 = np.ascontiguousarray(
        (2.0 * mu64).astype(np.float32).reshape(2, 128, K))
    m2 = (mu64 * mu64).sum(0)
    onesb = np.ones((1, 128), np.float32)
    negm2 = np.ascontiguousarray((-m2).astype(np.float32).reshape(1, K))
    gtab16 = np.ascontiguousarray(np.asarray(mu, np.float32).T).astype(
        np.float16)
    return mu2, onesb, negm2, gtab16


def kernel(images, mu, trace=False):
    from concourse import bass_utils

    images = np.asarray(images, np.float32)
    mu = np.asarray(mu, np.float32)

    if "nc" not in _CACHE:
        _CACHE["nc"] = _build_bass()
    nc = _CACHE["nc"]

    mu2, onesb, negm2, gtab16 = _prep_shared(mu)
    in_maps = []
    for i in range(NCORES):
        shard = images[i * BS:(i + 1) * BS]
        in_maps.append({
            "xt": np.ascontiguousarray(shard.T).reshape(2, 128, NT * 128),
            "mu2": mu2,
            "onesb": onesb,
            "negm2": negm2,
            "gtab16": gtab16,
        })

    res = bass_utils.run_bass_kernel_spmd(
        nc, in_maps, core_ids=list(range(NCORES)), trace=trace
    )
    _CACHE["last_results"] = res

    # Assemble fp16 recon -> f32, and collect per-row top-2 gap + argmax.
    out = np.empty((B_FULL, G), np.float32)
    gap = np.empty(B_FULL, np.float32)
    kdev = np.empty(B_FULL, np.int64)
    for c in range(NCORES):
        r = res.results[c]
        out[c * BS:(c + 1) * BS] = r["out16"].astype(np.float32)
        mx = r["mxv"]     # [NT//8, 128, 8tiles, 8]
        ix = r["idxv"]
        for j8 in range(NT // 8):
            for i in range(8):
                j = j8 * 8 + i
                rows = c * BS + j * 128 + np.arange(128)
                gap[rows] = mx[j8, :, i, 0] - mx[j8, :, i, 1]
                kdev[rows] = ix[j8, :, i, 0]

    # Host patch: exactly rescore rows whose device top-2 gap is within the
    # fp32r noise band; fixes any argmax flips the approximate scores caused.
    sus = np.where(gap < TAU)[0]
    _CACHE["n_patched"] = len(sus)
    if len(sus):
        x64 = images[sus].astype(np.float64)
        mu64 = mu.astype(np.float64)
        mu2c = (2.0 * mu64).astype(np.float32).astype(np.float64)
        m2c = (mu64 * mu64).sum(0).astype(np.float32).astype(np.float64)
        s = x64 @ mu2c - m2c
        kex = s.argmax(1)
        out[sus] = mu.T[kex]
    return out
